# revision 17
# baseline (speedup 1.0000x reference)
"""Trainium2 Bass kernel for a KAN layer (512->512, cubic B-spline, 17 ctrl pts).

Math: out[b,o] = sum_i w_b[i,o]*silu(xt[i,b]) + sum_i sum_c D[i,o,c]*B3_c(v[i,b])
with xt = clip(x.T, -bound, bound), v = (xt-g0)/h, D = w_s[:,:,None]*control_points.

The cubic B-spline basis over a uniform grid is rewritten via the truncated-power
identity  N3(s) = (1/6) * sum_m (-1)^m C(4,m) relu(s-m)^3, so the whole layer
collapses into ONE GEMM over K = 1 + 9*512 rows:
  [u | silu | u^2 | u^3 | relu(t-k3)^3 .. relu(t-k7)^3 | const]
against host-folded weights [G1 | w_b | G2 | G3 | E3..E7 | Gsum0 | ones].
Relu^3 pieces with knots below the clip range never truncate and fold into the
centered global cubic (G*); pieces with knots above it vanish.

Sharding: data-parallel over batch, 512 rows per core x 8 cores.

v2 restructure (vs the serial baseline): everything is chunked so the PE matmul
stream starts ~3us in and never stalls:
  - x arrives bf16 partition-major in 2 chunks, weights in 5 k-ordered chunks,
    all on the SP HWDGE ring in the order [w0, x0, x1, w1..w4] so the first
    matmul's inputs land first.
  - features are computed per 512-col chunk, spread across DVE (clip/relu/cubes),
    ACT (silu/squares) and Pool (2 knots' squares), in the same block order the
    matmul stream consumes them.
  - the matmul stream (k-tile outer, 4 batch-tiles inner, fp32 PSUM accumulate
    in 4 banks) chases the weight DMA and feature producers.
  - PSUM banks drain through parallel copies on ACT/DVE/Pool, one output DMA.
"""

import os
import sys

import numpy as np

for _p in ("/opt/trn_rl_repo",):
    if os.path.isdir(_p) and _p not in sys.path:
        sys.path.insert(0, _p)

BATCH, IN_DIM, OUT_DIM, NCORES = 4096, 512, 512, 8
BC = BATCH // NCORES  # 512 batch rows per core
NBLK = 9  # feature blocks of 512 K-rows each
NKT = NBLK * 4 + 1  # 37 k-tiles: 36 feature tiles + 1 const tile
NW = NKT + 1  # 38 weight tiles: 36 blocks + Gsum0 row-tile + ones tile

_nc_cache: dict = {}


def _build_nc(g0: float, h: float, bound: float):
    import concourse.bass as bass
    import concourse.mybir as mybir
    import concourse.tile as tile

    f32 = mybir.dt.float32
    bf16 = mybir.dt.bfloat16
    AF = mybir.ActivationFunctionType
    ALU = mybir.AluOpType

    tctr = g0 + 5.0 * h  # data-range center in t-units (0.0 for the default grid)
    knots = [g0 + k * h for k in range(3, 8)]

    nc = bass.Bass()
    x_d = nc.dram_tensor("xt", [128, 4, BC], bf16, kind="ExternalInput")
    w_d = nc.dram_tensor("w", [128, NW, OUT_DIM], bf16, kind="ExternalInput")
    out_d = nc.dram_tensor("out", [128, 4, OUT_DIM], bf16, kind="ExternalOutput")

    with tile.TileContext(nc) as tc:
        with (
            tc.tile_pool(name="data", bufs=1) as datap,
            tc.tile_pool(name="wt", bufs=1) as wp,
            tc.tile_pool(name="psum", bufs=1, space="PSUM") as pp,
        ):
            xt = datap.tile([128, 4, BC], bf16, name="xt_sb")
            wbig = wp.tile([128, NW, OUT_DIM], bf16, name="wbig")

            # SDMA engines round-robin across all active queues at packet
            # granularity, so concurrent chunks finish roughly together in
            # proportion to total queued bytes. x rides HWDGE (small, needed
            # first); the weight chunks go through SWDGE, whose single
            # software queue drains strictly FIFO -> k-ordered arrival the
            # matmul stream chases. The first chunk is half-size so the
            # stream starts sooner.
            nc.sync.dma_start(xt[:, 0:2, :], x_d[:, 0:2, :])
            nc.sync.dma_start(xt[:, 2:4, :], x_d[:, 2:4, :])
            for a, b in [(0, 2), (2, 4), (4, 8), (8, 16), (16, 28), (28, NW)]:
                nc.gpsimd.dma_start(wbig[:, a:b, :], w_d[:, a:b, :])

            _consts = {}

            def cbias(val: float):
                if val not in _consts:
                    ct = datap.tile([128, 1], f32, name=f"c{len(_consts)}")
                    nc.vector.memset(ct[:], val)
                    _consts[val] = ct
                return _consts[val][:]

            G = range(4)
            tc_t = datap.tile([128, 4, BC], bf16, name="tc")
            for g in G:
                nc.vector.tensor_scalar(
                    tc_t[:, g, :], xt[:, g, :], -bound, bound, ALU.max, ALU.min
                )

            if tctr == 0.0:
                u_t = tc_t
            else:
                u_t = datap.tile([128, 4, BC], bf16, name="u")
                for g in G:
                    nc.scalar.activation(u_t[:, g, :], tc_t[:, g, :], AF.Copy, bias=-tctr)

            # ACT: silu then the 5 relus (all value-stable activation ops, in
            # the order the matmul stream consumes the resulting blocks).
            silu_t = datap.tile([128, 4, BC], bf16, name="silu")
            for g in G:
                nc.scalar.activation(silu_t[:, g, :], tc_t[:, g, :], AF.Silu)
            r_ts = []
            for j, kn in enumerate(knots):
                r = datap.tile([128, 4, BC], bf16, name=f"r{j}")
                for g in G:
                    nc.scalar.activation(r[:, g, :], tc_t[:, g, :], AF.Relu, bias=cbias(-kn))
                r_ts.append(r)

            # DVE: squares and cubes (tensor_tensor muls are value-stable).
            # Knot 4's pair lives on Pool so its only foreign input is ACT's r.
            u2_t = datap.tile([128, 4, BC], bf16, name="u2")
            for g in G:
                nc.vector.tensor_mul(u2_t[:, g, :], u_t[:, g, :], u_t[:, g, :])
            u3_t = datap.tile([128, 4, BC], bf16, name="u3")
            for g in G:
                nc.vector.tensor_mul(u3_t[:, g, :], u2_t[:, g, :], u_t[:, g, :])

            POOL_KNOTS = {4}
            r3_ts = []
            for j, kn in enumerate(knots):
                eng = nc.gpsimd if j in POOL_KNOTS else nc.vector
                r2 = datap.tile([128, 4, BC], bf16, name=f"r2_{j}")
                for g in G:
                    eng.tensor_mul(r2[:, g, :], r_ts[j][:, g, :], r_ts[j][:, g, :])
                r3 = datap.tile([128, 4, BC], bf16, name=f"r3_{j}")
                for g in G:
                    eng.tensor_mul(r3[:, g, :], r2[:, g, :], r_ts[j][:, g, :])
                r3_ts.append(r3)

            feat_tiles = [u_t, silu_t, u2_t, u3_t] + r3_ts

            psums = [pp.tile([128, OUT_DIM], f32, name=f"ps{m}") for m in range(4)]
            for kt in range(NKT):
                for m in range(4):
                    if kt == NKT - 1:
                        # const: all-ones stationary x Gsum0 row-tile moving
                        lhsT = wbig[:, NW - 1, m * 128 : (m + 1) * 128]
                        rhs = wbig[:, NW - 2, :]
                    else:
                        blk, gi = kt // 4, kt % 4
                        lhsT = feat_tiles[blk][:, gi, m * 128 : (m + 1) * 128]
                        rhs = wbig[:, kt, :]
                    nc.tensor.matmul(
                        psums[m][:], lhsT, rhs, start=(kt == 0), stop=(kt == NKT - 1)
                    )

            # bf16 store halves the output DMA; the 2e-2 gate dwarfs the
            # ~4e-3 it costs. Host upcasts to float32.
            osb = datap.tile([128, 4, OUT_DIM], bf16, name="osb")
            nc.scalar.copy(osb[:, 0, :], psums[0][:])
            nc.vector.tensor_copy(osb[:, 1, :], psums[1][:])
            nc.vector.tensor_copy(osb[:, 2, :], psums[2][:])
            nc.scalar.copy(osb[:, 3, :], psums[3][:])
            nc.sync.dma_start(out_d[:], osb[:])

    # TPB instructions carry a single sync-wait slot; the Tile scheduler's
    # reordering leaves some cube muls with two waits. Run the same
    # legalization Bacc.compile uses to split them into event-semaphore ops.
    import bass_rust as _bass_rust

    _bass_rust.generate_event_semaphores(nc)

    # The Tile kernel-tail drain waits on every proc's sem (6 waits), but the
    # TPB Drain encoding holds fewer. All dataflow here funnels into the single
    # output-store DMA: its completion transitively implies PE/ACT/DVE/Pool and
    # the input DMAs finished, so keep only that queue's wait on the drain.
    import concourse.mybir as mybir

    out_q = None
    insts = []
    for bb in nc.m.functions[0].blocks:
        insts.extend(bb.instructions)
    for ins in insts:
        if type(ins).__name__ == "InstDMACopy" and ins.sync_info is not None:
            for u in ins.sync_info.on_update:
                if u.ant_name.startswith("DMAHW") or u.ant_name.startswith("DMASW"):
                    out_q = u.ant_name
    assert out_q is not None
    for ins in insts:
        if type(ins).__name__ == "InstDrain" and ins.sync_info is not None:
            kept = [w for w in ins.sync_info.on_wait if w.ant_name == out_q]
            ins.sync_info = mybir.SyncInfo(on_wait=kept, on_update=list(ins.sync_info.on_update))
    return nc


def _fold_weights(w_b, w_s, control_points, g0, h, bound):
    """Host-side fold: 17 control points -> 9 GEMM weight blocks (float64 math).

    Features are computed on-device in t-units (tc = clip(x), u = tc - tctr,
    r_k = relu(tc - knot_k)); the 1/h^j scalings fold into the weights here.
    Returns W laid out partition-major: [128, NW, OUT].
    """
    from math import comb

    D = w_s[:, :, None].astype(np.float64) * control_points.astype(np.float64)
    E = np.zeros((8, IN_DIM, OUT_DIM))
    for k in range(8):
        for c in range(max(0, k - 4), min(7, k) + 1):
            E[k] += D[:, :, c] * ((-1.0) ** (k - c) * comb(4, k - c) / 6.0)

    ctr = 5.0  # v-space center of the clipped data range [2.5, 7.5]
    a = [ctr - 0.0, ctr - 1.0, ctr - 2.0]
    G3 = E[0] + E[1] + E[2]
    G2 = 3.0 * (a[0] * E[0] + a[1] * E[1] + a[2] * E[2])
    G1 = 3.0 * (a[0] ** 2 * E[0] + a[1] ** 2 * E[1] + a[2] ** 2 * E[2])
    G0 = a[0] ** 3 * E[0] + a[1] ** 3 * E[1] + a[2] ** 3 * E[2]
    Gsum0 = G0.sum(axis=0)

    W = np.zeros((NW, 128, OUT_DIM), np.float32)
    W[NW - 1] = 1.0  # ones tile (stationary side of the const matmul)
    W[NW - 2, 0, :] = Gsum0.astype(np.float32)
    blocks = [G1 / h, w_b.astype(np.float64), G2 / h**2, G3 / h**3] + [
        E[k] / h**3 for k in range(3, 8)
    ]
    for bi, blk in enumerate(blocks):
        W[bi * 4 : (bi + 1) * 4] = blk.reshape(4, 128, OUT_DIM).astype(np.float32)
    return np.ascontiguousarray(W.transpose(1, 0, 2))


last_results = None


def kernel(x, w_b, w_s, control_points, grid_points, bound):
    global last_results
    import ml_dtypes

    x = np.asarray(x, np.float32)
    w_b = np.asarray(w_b, np.float32)
    w_s = np.asarray(w_s, np.float32)
    control_points = np.asarray(control_points, np.float32)
    grid_points = np.asarray(grid_points, np.float64)
    bound = float(np.asarray(bound))

    g0 = float(grid_points[0])
    h = float((grid_points[-1] - grid_points[0]) / (len(grid_points) - 1))

    W = _fold_weights(w_b, w_s, control_points, g0, h, bound).astype(ml_dtypes.bfloat16)

    key = (g0, h, bound)
    if key not in _nc_cache:
        _nc_cache[key] = _build_nc(g0, h, bound)
    nc = _nc_cache[key]

    in_maps = []
    for k in range(NCORES):
        xk = x[k * BC : (k + 1) * BC, :].T.reshape(4, 128, BC).transpose(1, 0, 2)
        xk = np.ascontiguousarray(xk.astype(ml_dtypes.bfloat16))
        in_maps.append({"xt": xk, "w": W})

    from concourse.bass_utils import run_bass_kernel_spmd

    last_results = run_bass_kernel_spmd(nc, in_maps, list(range(NCORES)))
    out = np.concatenate(
        [
            np.asarray(last_results.results[k]["out"], dtype=np.float32)
            .transpose(1, 0, 2)
            .reshape(BC, OUT_DIM)
            for k in range(NCORES)
        ],
        axis=0,
    )
    return out


# revision 19
# speedup vs baseline: 1.1799x; 1.1799x over previous
"""Trainium2 Bass kernel for a KAN layer (512->512, cubic B-spline, 17 ctrl pts).

Math: out[b,o] = sum_i w_b[i,o]*silu(xt[i,b]) + sum_i sum_c D[i,o,c]*B3_c(v[i,b])
with xt = clip(x.T, -bound, bound), v = (xt-g0)/h, D = w_s[:,:,None]*control_points.

The cubic B-spline basis over a uniform grid is rewritten via the truncated-power
identity  N3(s) = (1/6) * sum_m (-1)^m C(4,m) relu(s-m)^3, so the whole layer
collapses into ONE GEMM over K = 1 + 9*512 rows:
  [u | silu | u^3 | u^2 | relu(t-k3)^3 .. relu(t-k7)^3 | const]
against host-folded weights. Relu^3 pieces with knots below the clip range never
truncate and fold into the centered global cubic; pieces above it vanish.

Sharding: data-parallel over batch, 512 rows per core x 8 cores.

Mixed precision: the u/silu/u^3/const blocks stay bf16; the u^2 and five relu^3
blocks run as fp8e4 DoubleRow matmuls (2 contraction rows per PE cell, ~1.4x).
Scales are all powers of two folded host-side: every weight is premultiplied by
S=4096 (host divides the output by S), the relu features are computed as r/2
(ACT scale) so r^3/8 fits fp8's +-240 range, and the fp8 weight blocks carry
the compensating 8S. No extra on-device ops are spent on scaling.

Dataflow: x rides HWDGE; weights stream through SWDGE whose single software
queue drains strictly FIFO -> k-ordered arrival the matmul stream chases.
Features are computed per 512-col chunk on ACT (silu/relus) + DVE (cubes) +
Pool (last knot), in consumption order. PSUM drains via parallel ACT/DVE
copies to bf16 and one output DMA.
"""

import os
import sys

import numpy as np

for _p in ("/opt/trn_rl_repo",):
    if os.path.isdir(_p) and _p not in sys.path:
        sys.path.insert(0, _p)

BATCH, IN_DIM, OUT_DIM, NCORES = 4096, 512, 512, 8
BC = BATCH // NCORES  # 512 batch rows per core
S = 4096.0  # global dequant scale (pow2); host divides output by S
NBF = 14  # bf16 weight tiles: u 0:4, silu 4:8, u3 8:12, Gsum 12, ones 13
NF8 = 24  # fp8 weight tiles: u2 0:4, r3_j 4+4j:8+4j

_nc_cache: dict = {}


def _build_nc(g0: float, h: float, bound: float):
    import concourse.bass as bass
    import concourse.mybir as mybir
    import concourse.tile as tile

    f32 = mybir.dt.float32
    bf16 = mybir.dt.bfloat16
    f8 = mybir.dt.float8e4
    AF = mybir.ActivationFunctionType
    ALU = mybir.AluOpType
    DR = mybir.MatmulPerfMode.DoubleRow

    tctr = g0 + 5.0 * h  # data-range center in t-units (0.0 for the default grid)
    knots = [g0 + k * h for k in range(3, 8)]

    nc = bass.Bass()
    x_d = nc.dram_tensor("xt", [128, 4, BC], bf16, kind="ExternalInput")
    wbf_d = nc.dram_tensor("wbf", [128, NBF, OUT_DIM], bf16, kind="ExternalInput")
    wf8_d = nc.dram_tensor("wf8", [128, NF8, OUT_DIM], f8, kind="ExternalInput")
    out_d = nc.dram_tensor("out", [128, 4, OUT_DIM], bf16, kind="ExternalOutput")

    with tile.TileContext(nc) as tc:
        with (
            tc.tile_pool(name="data", bufs=1) as datap,
            tc.tile_pool(name="wt", bufs=1) as wp,
            tc.tile_pool(name="psum", bufs=1, space="PSUM") as pp,
        ):
            xt = datap.tile([128, 4, BC], bf16, name="xt_sb")
            wbf = wp.tile([128, NBF, OUT_DIM], bf16, name="wbf_sb")
            wf8 = wp.tile([128, NF8, OUT_DIM], f8, name="wf8_sb")

            nc.sync.dma_start(xt[:, 0:2, :], x_d[:, 0:2, :])
            nc.sync.dma_start(xt[:, 2:4, :], x_d[:, 2:4, :])
            # SWDGE FIFO, in stream-consumption order.
            for t_, a, b in [
                ("bf", 0, 2), ("bf", 2, 4), ("bf", 4, 8), ("bf", 8, 12),
                ("f8", 0, 4), ("f8", 4, 12), ("f8", 12, 24), ("bf", 12, 14),
            ]:
                sb, dr = (wbf, wbf_d) if t_ == "bf" else (wf8, wf8_d)
                nc.gpsimd.dma_start(sb[:, a:b, :], dr[:, a:b, :])

            _consts = {}

            def cbias(val: float):
                if val not in _consts:
                    ct = datap.tile([128, 1], f32, name=f"c{len(_consts)}")
                    nc.vector.memset(ct[:], val)
                    _consts[val] = ct
                return _consts[val][:]

            G = range(4)
            tc_t = datap.tile([128, 4, BC], bf16, name="tc")
            for g in G:
                nc.vector.tensor_scalar(
                    tc_t[:, g, :], xt[:, g, :], -bound, bound, ALU.max, ALU.min
                )

            if tctr == 0.0:
                u_t = tc_t
            else:
                u_t = datap.tile([128, 4, BC], bf16, name="u")
                for g in G:
                    nc.scalar.activation(u_t[:, g, :], tc_t[:, g, :], AF.Copy, bias=-tctr)

            # ACT: silu, then the relus as r/2 (scale folded into fp8 weights).
            silu_t = datap.tile([128, 4, BC], bf16, name="silu")
            for g in G:
                nc.scalar.activation(silu_t[:, g, :], tc_t[:, g, :], AF.Silu)
            r_ts = []
            for j, kn in enumerate(knots):
                r = datap.tile([128, 4, BC], bf16, name=f"r{j}")
                for g in G:
                    nc.scalar.activation(
                        r[:, g, :], tc_t[:, g, :], AF.Relu,
                        bias=cbias(-kn / 2), scale=0.5,
                    )
                r_ts.append(r)

            # DVE: cubes (value-stable tensor_tensor muls).
            u2_t = datap.tile([128, 4, BC], bf16, name="u2")
            for g in G:
                nc.vector.tensor_mul(u2_t[:, g, :], u_t[:, g, :], u_t[:, g, :])
            u3_t = datap.tile([128, 4, BC], bf16, name="u3")
            for g in G:
                nc.vector.tensor_mul(u3_t[:, g, :], u2_t[:, g, :], u_t[:, g, :])
            u2f8_t = datap.tile([128, 4, BC], f8, name="u2f8")
            for g in G:
                nc.vector.tensor_mul(u2f8_t[:, g, :], u_t[:, g, :], u_t[:, g, :])

            # Knot 4's square runs on ACT: Square(t/2 - kn/2) equals (r/2)^2
            # wherever r3 = r2*r is nonzero, so the unclipped square works.
            # Pool's software fp8 TTs were the stream's last stall source.
            r3_ts = []
            for j, kn in enumerate(knots):
                r2 = datap.tile([128, 4, BC], bf16, name=f"r2_{j}")
                for g in G:
                    if j == 4:
                        nc.scalar.activation(
                            r2[:, g, :], tc_t[:, g, :], AF.Square,
                            bias=cbias(-kn / 2), scale=0.5,
                        )
                    else:
                        nc.vector.tensor_mul(r2[:, g, :], r_ts[j][:, g, :], r_ts[j][:, g, :])
                r3 = datap.tile([128, 4, BC], f8, name=f"r3_{j}")
                for g in G:
                    nc.vector.tensor_mul(r3[:, g, :], r2[:, g, :], r_ts[j][:, g, :])
                r3_ts.append(r3)

            # Matmul stream: bf16 blocks one k-tile at a time, fp8 blocks as
            # DoubleRow pairs, const block last; 4 batch-tiles inner.
            steps = []
            for blk, ft in enumerate([u_t, silu_t, u3_t]):
                for gi in G:
                    steps.append(("bf", ft, gi, blk * 4 + gi))
            for fi, ft in enumerate([u2f8_t] + r3_ts):
                for q in range(2):
                    steps.append(("f8", ft, 2 * q, fi * 4 + 2 * q))
            steps.append(("const", None, 0, 0))

            psums = [pp.tile([128, OUT_DIM], f32, name=f"ps{m}") for m in range(4)]
            last = len(steps) - 1
            for si, (kind, ft, gi, wi) in enumerate(steps):
                for m in range(4):
                    ms = slice(m * 128, (m + 1) * 128)
                    if kind == "bf":
                        nc.tensor.matmul(
                            psums[m][:], ft[:, gi, ms], wbf[:, wi, :],
                            start=(si == 0), stop=(si == last),
                            skip_group_check=True,
                        )
                    elif kind == "f8":
                        nc.tensor.matmul(
                            psums[m][:], ft[:, gi : gi + 2, ms], wf8[:, wi : wi + 2, :],
                            start=(si == 0), stop=(si == last),
                            perf_mode=DR, skip_group_check=True,
                        )
                    else:  # const: all-ones stationary x (S*Gsum0) row-tile
                        nc.tensor.matmul(
                            psums[m][:], wbf[:, NBF - 1, ms], wbf[:, NBF - 2, :],
                            start=(si == 0), stop=(si == last),
                            skip_group_check=True,
                        )

            osb = datap.tile([128, 4, OUT_DIM], bf16, name="osb")
            nc.scalar.copy(osb[:, 0, :], psums[0][:])
            nc.vector.tensor_copy(osb[:, 1, :], psums[1][:])
            nc.vector.tensor_copy(osb[:, 2, :], psums[2][:])
            nc.scalar.copy(osb[:, 3, :], psums[3][:])
            nc.sync.dma_start(out_d[:], osb[:])

    # TPB instructions carry a single sync-wait slot; split multi-waits the
    # same way Bacc.compile does.
    import bass_rust as _bass_rust

    _bass_rust.generate_event_semaphores(nc)

    # Keep only the output-store queue's wait on the kernel-tail drains (its
    # completion transitively implies everything else finished).
    import concourse.mybir as mybir

    out_q = None
    insts = []
    for bb in nc.m.functions[0].blocks:
        insts.extend(bb.instructions)
    for ins in insts:
        if type(ins).__name__ == "InstDMACopy" and ins.sync_info is not None:
            for u in ins.sync_info.on_update:
                if u.ant_name.startswith("DMAHW") or u.ant_name.startswith("DMASW"):
                    out_q = u.ant_name
    assert out_q is not None
    for ins in insts:
        if type(ins).__name__ == "InstDrain" and ins.sync_info is not None:
            kept = [w for w in ins.sync_info.on_wait if w.ant_name == out_q]
            ins.sync_info = mybir.SyncInfo(on_wait=kept, on_update=list(ins.sync_info.on_update))
    return nc


def _fold_weights(w_b, w_s, control_points, g0, h, bound):
    """Host-side fold (float64): control points -> GEMM weight blocks.

    Returns (Wbf [128,NBF,OUT] f32, Wf8 [128,NF8,OUT] f32) with all scales
    (global S, the r/2 feature halving, per-dtype placement) pre-applied.
    """
    from math import comb

    D = w_s[:, :, None].astype(np.float64) * control_points.astype(np.float64)
    E = np.zeros((8, IN_DIM, OUT_DIM))
    for k in range(8):
        for c in range(max(0, k - 4), min(7, k) + 1):
            E[k] += D[:, :, c] * ((-1.0) ** (k - c) * comb(4, k - c) / 6.0)

    ctr = 5.0  # v-space center of the clipped data range [2.5, 7.5]
    a = [ctr - 0.0, ctr - 1.0, ctr - 2.0]
    G3 = E[0] + E[1] + E[2]
    G2 = 3.0 * (a[0] * E[0] + a[1] * E[1] + a[2] * E[2])
    G1 = 3.0 * (a[0] ** 2 * E[0] + a[1] ** 2 * E[1] + a[2] ** 2 * E[2])
    G0 = a[0] ** 3 * E[0] + a[1] ** 3 * E[1] + a[2] ** 3 * E[2]
    Gsum0 = G0.sum(axis=0)

    Wbf = np.zeros((NBF, 128, OUT_DIM), np.float32)
    for bi, blk in enumerate([G1 / h * S, w_b.astype(np.float64) * S, G3 / h**3 * S]):
        Wbf[bi * 4 : (bi + 1) * 4] = blk.reshape(4, 128, OUT_DIM).astype(np.float32)
    Wbf[NBF - 2, 0, :] = (Gsum0 * S).astype(np.float32)
    Wbf[NBF - 1] = 1.0

    Wf8 = np.zeros((NF8, 128, OUT_DIM), np.float32)
    Wf8[0:4] = (G2 / h**2 * S).reshape(4, 128, OUT_DIM).astype(np.float32)
    for j in range(5):
        blk = E[3 + j] / h**3 * (8.0 * S)  # features are (r/2)^3 = r^3/8
        Wf8[4 + 4 * j : 8 + 4 * j] = blk.reshape(4, 128, OUT_DIM).astype(np.float32)
    amax = np.abs(Wf8).max()
    assert amax <= 232.0, f"fp8 weight overflow: {amax}"
    return (
        np.ascontiguousarray(Wbf.transpose(1, 0, 2)),
        np.ascontiguousarray(Wf8.transpose(1, 0, 2)),
    )


last_results = None


def kernel(x, w_b, w_s, control_points, grid_points, bound):
    global last_results
    import ml_dtypes

    x = np.asarray(x, np.float32)
    w_b = np.asarray(w_b, np.float32)
    w_s = np.asarray(w_s, np.float32)
    control_points = np.asarray(control_points, np.float32)
    grid_points = np.asarray(grid_points, np.float64)
    bound = float(np.asarray(bound))

    g0 = float(grid_points[0])
    h = float((grid_points[-1] - grid_points[0]) / (len(grid_points) - 1))

    Wbf, Wf8 = _fold_weights(w_b, w_s, control_points, g0, h, bound)
    Wbf = Wbf.astype(ml_dtypes.bfloat16)
    Wf8 = Wf8.astype(ml_dtypes.float8_e4m3)

    key = (g0, h, bound)
    if key not in _nc_cache:
        _nc_cache[key] = _build_nc(g0, h, bound)
    nc = _nc_cache[key]

    in_maps = []
    for k in range(NCORES):
        xk = x[k * BC : (k + 1) * BC, :].T.reshape(4, 128, BC).transpose(1, 0, 2)
        xk = np.ascontiguousarray(xk.astype(ml_dtypes.bfloat16))
        in_maps.append({"xt": xk, "wbf": Wbf, "wf8": Wf8})

    from concourse.bass_utils import run_bass_kernel_spmd

    last_results = run_bass_kernel_spmd(nc, in_maps, list(range(NCORES)))
    out = np.concatenate(
        [
            (np.asarray(last_results.results[k]["out"], dtype=np.float32) / S)
            .transpose(1, 0, 2)
            .reshape(BC, OUT_DIM)
            for k in range(NCORES)
        ],
        axis=0,
    )
    return out
